# revision 1
# baseline (speedup 1.0000x reference)
"""Trainium2 Bass kernel for nn_CrossScalePeriodicFeatureAggregator.

Reference computation (per expert e with patch size p_e, L_e = 336 / p_e):
    h = einsum('nld,pd->nlp', xs_e, W_e) + b_e      # [128, L_e, p_e*512]
    h -> reshape [128, 336, 512]                     # seq-stitch
    proj = h @ Wp.T + bp                             # shared projection
    out[batch_index] += gate * proj                  # gated scatter-combine

Kernel strategy (8 cores, SPMD):
  * Algebraic fusion: the two chained matmuls collapse into one. For output
    position s = l*p_e + q:  out[n, s, :] = x[n, l, :] @ WF_e[q]  where
    WF_e[q] = W_e[q*512:(q+1)*512, :].T @ Wp.T   (precomputed on host).
    This halves device FLOPs (90 GF instead of 180 GF).
  * Gates are folded into x rows on host (mathematically identical).
  * Token sharding: core c takes rows [16c, 16c+16) of every expert
    -> perfectly balanced (each core: 16*336 tokens per expert through a
    512->512 matmul per q), single uniform SPMD program.
  * Matmuls run in float32r (TF32-like, full PE rate at N=512, measured
    rel-err ~1.4e-4 vs the fp32 reference for K=512 contractions).
  * Device writes per-expert projections out_e[q, t, :] (q-major, large
    contiguous DMAs); host de-interleaves and does the gated combine.
"""
import numpy as np

PATCH = [4, 8, 12, 24]
SEQ = 336
D = 512
NE = 4
BATCH = 256
ROWS_PER_EXPERT = 128
N_CORES = 8
ROWS_PER_CORE = ROWS_PER_EXPERT // N_CORES          # 16
L = [SEQ // p for p in PATCH]                       # [84, 42, 28, 14]
T = [ROWS_PER_CORE * l for l in L]                  # tokens/core: [1344, 672, 448, 224]
NT = [(t + 127) // 128 for t in T]                  # token tiles: [11, 6, 4, 2]
KC = 4                                              # contraction chunks of 128

_CACHED = {}


def _job_order(interleave):
    if not interleave:
        return [(e, q) for e in range(NE) for q in range(PATCH[e])]
    # merge experts by fractional position: pairs DMA-heavy (e3) jobs with
    # compute-heavy (e0) jobs to smooth HBM read demand
    jobs = [((q + 0.5) / PATCH[e], e, q)
            for e in range(NE) for q in range(PATCH[e])]
    return [(e, q) for _, e, q in sorted(jobs)]


def _build_nc(loop_n=0, internal_wf=False, internal_out=False,
              wbufs=8, sbufs=4, interleave=True, copies="dve", rings="a"):
    """loop_n>0 wraps the compute body in a hardware For_i loop (differential
    HW timing); internal_wf/internal_out source weights from / sink outputs to
    internal DRAM so host transfer stays tiny (timing builds only)."""
    import concourse.mybir as mybir
    from concourse import bacc
    from concourse.tile import TileContext

    f32r = mybir.dt.float32r
    f32 = mybir.dt.float32

    nc = bacc.Bacc("TRN2", target_bir_lowering=False, debug=False,
                   num_devices=N_CORES)
    xt = [nc.dram_tensor(f"xt{e}", [128, KC * T[e]], f32r, kind="ExternalInput")
          for e in range(NE)]
    if internal_wf:
        wf = [nc.dram_tensor(f"iwf{e}", [PATCH[e], 128, KC * D], f32r)
              for e in range(NE)]
    else:
        wf = [nc.dram_tensor(f"wf{e}", [PATCH[e], 128, KC * D], f32r,
                             kind="ExternalInput") for e in range(NE)]
    # partition-major layout [q, j, mt*D]: token t = mt*128 + j, so every
    # SBUF partition writes one contiguous run. Tail tokens >= T[e] are
    # garbage from the partial tile and are sliced off on the host.
    if internal_out:
        out = [nc.dram_tensor(f"io{e}", [PATCH[e], 128, NT[e] * D], f32)
               for e in range(NE)]
        tiny = nc.dram_tensor("tiny", [128, D], f32r, kind="ExternalOutput")
    else:
        out = [nc.dram_tensor(f"out{e}", [PATCH[e], 128, NT[e] * D], f32,
                              kind="ExternalOutput") for e in range(NE)]

    MAXNT = max(NT)
    with TileContext(nc) as tc:
        with (
            tc.tile_pool(name="xpool", bufs=1) as xpool,
            tc.tile_pool(name="wpool", bufs=wbufs) as wpool,
            tc.tile_pool(name="spool", bufs=sbufs) as spool,
            tc.tile_pool(name="ppool", bufs=8, space="PSUM") as ppool,
        ):
            xtiles = []
            for e in range(NE):
                t = xpool.tile([128, KC * T[e]], f32r, tag=f"xt{e}")
                nc.sync.dma_start(t[:], xt[e].ap())
                xtiles.append(t)

            # weights and output stores ride separate DMA rings so stores
            # never block weight prefetch (FIFO per ring).
            if rings == "a":      # w: sync | out: scalar+gpsimd
                w_engs = [nc.sync]
                out_engs = [nc.scalar, nc.gpsimd]
            else:                 # w: sync+scalar | out: gpsimd
                w_engs = [nc.sync, nc.scalar]
                out_engs = [nc.gpsimd]
            state = {"flip": 0, "oi": 0}

            def body():
                for e, q in _job_order(interleave):
                        wt = wpool.tile([128, KC * D], f32r, tag="wt")
                        w_engs[state["oi"] % len(w_engs)].dma_start(
                            wt[:], wf[e].ap()[q])
                        st = spool.tile([128, MAXNT * D], f32, tag="st")
                        for mt in range(NT[e]):
                            m = min(128, T[e] - 128 * mt)
                            ps = ppool.tile([128, D], f32)
                            for k in range(KC):
                                nc.tensor.matmul(
                                    ps[:m, :],
                                    xtiles[e][:, k * T[e] + 128 * mt:
                                              k * T[e] + 128 * mt + m],
                                    wt[:, k * D:(k + 1) * D],
                                    start=(k == 0), stop=(k == KC - 1),
                                )
                            dst = st[:m, mt * D:(mt + 1) * D]
                            if copies == "dve" or state["flip"] % 2:
                                nc.vector.tensor_copy(dst, ps[:m, :])
                            else:
                                nc.scalar.copy(dst, ps[:m, :])
                            state["flip"] += 1
                        # one fully-contiguous DMA for this (e, q)
                        out_engs[state["oi"] % len(out_engs)].dma_start(
                            out[e].ap()[q], st[:, :NT[e] * D])
                        state["oi"] += 1

            if loop_n > 0:
                with tc.For_i(0, loop_n, 1):
                    body()
            else:
                body()
            if internal_out:
                nc.sync.dma_start(tiny.ap(), xtiles[0][:, :D])
    nc.compile()
    return nc


def _get_nc():
    if "nc" not in _CACHED:
        _CACHED["nc"] = _build_nc()
    return _CACHED["nc"]


def _prep(xs, Ws, gates, Wp, batch_index, expert_index):
    """Host-side shard prep. Returns (in_maps, row_of_expert, g_row)."""
    row_of_expert = [np.nonzero(expert_index == e)[0] for e in range(NE)]
    g_row = gates[batch_index, expert_index].astype(np.float32)   # [NNZ]

    # Fused weights WF_e[q] = W_e[q*512:(q+1)*512, :].T @ Wp.T  -> [c, d_out];
    # device layout wf_e[q, p, k*512+d] with c = 128k + p.
    wf_in = []
    for e in range(NE):
        p = PATCH[e]
        w = Ws[e].reshape(p, D, D)                     # [q, d_mid, c]
        WF = np.einsum("qdc,od->qco", w, Wp, optimize=True)   # [q, c, d_out]
        wf_in.append(np.ascontiguousarray(
            WF.reshape(p, KC, 128, D).transpose(0, 2, 1, 3)   # [q, p128, k, d]
              .reshape(p, 128, KC * D)))

    in_maps = []
    for c in range(N_CORES):
        m = {}
        for e in range(NE):
            rows = slice(c * ROWS_PER_CORE, (c + 1) * ROWS_PER_CORE)
            gr = g_row[row_of_expert[e][rows]]
            x = xs[e][rows] * gr[:, None, None]        # [16, L, 512]
            x = x.reshape(T[e], D)                     # tokens
            # xt[p, k*T + t] = x[t, 128k + p]
            m[f"xt{e}"] = np.ascontiguousarray(
                x.reshape(T[e], KC, 128).transpose(2, 1, 0)
                 .reshape(128, KC * T[e]))
            m[f"wf{e}"] = wf_in[e]
        in_maps.append(m)
    return in_maps, row_of_expert, g_row


def _combine(results, row_of_expert, batch_index):
    """De-interleave q-major device outputs and gated-combine per batch."""
    combined = np.zeros((BATCH, SEQ, D), np.float32)
    for e in range(NE):
        p = PATCH[e]
        full = np.empty((ROWS_PER_EXPERT, SEQ, D), np.float32)
        for c in range(N_CORES):
            # device layout [q, j, mt, d]; token t = mt*128 + j
            raw = results[c][f"out{e}"].reshape(p, 128, NT[e], D)
            dev = raw.transpose(0, 2, 1, 3).reshape(p, NT[e] * 128, D)[:, :T[e], :]
            # out_seq[r, l*p + q, :] = dev[q, r*L + l, :]
            blk = dev.reshape(p, ROWS_PER_CORE, L[e], D).transpose(1, 2, 0, 3)
            full[c * ROWS_PER_CORE:(c + 1) * ROWS_PER_CORE] = \
                blk.reshape(ROWS_PER_CORE, SEQ, D)
        bids = batch_index[row_of_expert[e]]
        if len(np.unique(bids)) == len(bids):
            combined[bids] += full
        else:
            np.add.at(combined, bids, full)
    return combined


def kernel(xs0, xs1, xs2, xs3, gates, W0, b0, W1, b1, W2, b2, W3, b3, Wp, bp,
           batch_index, expert_index, _collect_results=None):
    from concourse.bass_utils import run_bass_kernel_spmd

    xs = [np.asarray(x, np.float32) for x in (xs0, xs1, xs2, xs3)]
    Ws = [np.asarray(w, np.float32) for w in (W0, W1, W2, W3)]
    bs = [np.asarray(b, np.float32) for b in (b0, b1, b2, b3)]
    gates = np.asarray(gates, np.float32)
    Wp = np.asarray(Wp, np.float32)
    bp = np.asarray(bp, np.float32)
    batch_index = np.asarray(batch_index)
    expert_index = np.asarray(expert_index)

    in_maps, row_of_expert, g_row = _prep(xs, Ws, gates, Wp,
                                          batch_index, expert_index)
    nc = _get_nc()
    res = run_bass_kernel_spmd(nc, in_maps, list(range(N_CORES)))
    if _collect_results is not None:
        _collect_results.append(res)

    combined = _combine(res.results, row_of_expert, batch_index)

    # Bias terms (zero in this problem's inputs; handled for correctness).
    if any(np.any(b) for b in bs) or np.any(bp):
        for e in range(NE):
            p = PATCH[e]
            bF = bs[e].reshape(p, D) @ Wp.T + bp       # [q, d_out]
            bias_seq = np.tile(bF, (L[e], 1)).reshape(SEQ, D)
            bids = batch_index[row_of_expert[e]]
            gr = g_row[row_of_expert[e]]
            contrib = gr[:, None, None] * bias_seq[None]
            if len(np.unique(bids)) == len(bids):
                combined[bids] += contrib
            else:
                np.add.at(combined, bids, contrib)

    return combined



# revision 12
# speedup vs baseline: 1.2600x; 1.2600x over previous
"""Trainium2 Bass kernel for nn_CrossScalePeriodicFeatureAggregator.

Reference computation (per expert e with patch size p_e, L_e = 336 / p_e):
    h = einsum('nld,pd->nlp', xs_e, W_e) + b_e      # [128, L_e, p_e*512]
    h -> reshape [128, 336, 512]                     # seq-stitch
    proj = h @ Wp.T + bp                             # shared projection
    out[batch_index] += gate * proj                  # gated scatter-combine

Kernel strategy (8 cores, SPMD):
  * Algebraic fusion: the two chained matmuls collapse into one. For output
    position s = l*p_e + q:  out[n, s, :] = x[n, l, :] @ WF_e[q]  where
    WF_e[q] = W_e[q*512:(q+1)*512, :].T @ Wp.T   (precomputed on host).
    This halves device FLOPs (90 GF instead of 180 GF).
  * Gates are folded into x rows on host (mathematically identical).
  * Token sharding: core c takes rows [16c, 16c+16) of every expert
    -> perfectly balanced (each core: 16*336 tokens per expert through a
    512->512 matmul per q), single uniform SPMD program.
  * Matmuls run in float32r (TF32-like, full PE rate at N=512, measured
    rel-err ~1.4e-4 vs the fp32 reference for K=512 contractions).
  * Device writes per-expert projections out_e[q, t, :] (q-major, large
    contiguous DMAs); host de-interleaves and does the gated combine.
"""
import numpy as np

PATCH = [4, 8, 12, 24]
SEQ = 336
D = 512
NE = 4
BATCH = 256
ROWS_PER_EXPERT = 128
N_CORES = 8
ROWS_PER_CORE = ROWS_PER_EXPERT // N_CORES          # 16
L = [SEQ // p for p in PATCH]                       # [84, 42, 28, 14]
T = [ROWS_PER_CORE * l for l in L]                  # tokens/core: [1344, 672, 448, 224]
NT = [(t + 127) // 128 for t in T]                  # token tiles: [11, 6, 4, 2]
KC = 4                                              # contraction chunks of 128
DT = "f32r"                                         # "f32r" | "bf16" device dtype (v1)
V = 2                                               # active kernel version

_CACHED = {}


def _job_order(interleave):
    if not interleave:
        return [(e, q) for e in range(NE) for q in range(PATCH[e])]
    # merge experts by fractional position: pairs DMA-heavy (e3) jobs with
    # compute-heavy (e0) jobs to smooth HBM read demand
    jobs = [((q + 0.5) / PATCH[e], e, q)
            for e in range(NE) for q in range(PATCH[e])]
    return [(e, q) for _, e, q in sorted(jobs)]


def _build_nc(loop_n=0, internal_wf=False, internal_out=False,
              wbufs=8, sbufs=4, interleave=True, copies="dve", rings="a",
              dt="f32r", out_dt=None):
    """loop_n>0 wraps the compute body in a hardware For_i loop (differential
    HW timing); internal_wf/internal_out source weights from / sink outputs to
    internal DRAM so host transfer stays tiny (timing builds only)."""
    import concourse.mybir as mybir
    from concourse import bacc
    from concourse.tile import TileContext

    f32r = mybir.dt.float32r if dt == "f32r" else mybir.dt.bfloat16
    f32 = mybir.dt.float32 if (out_dt or dt) == "f32r" else mybir.dt.bfloat16
    psum_dt = mybir.dt.float32

    nc = bacc.Bacc("TRN2", target_bir_lowering=False, debug=False,
                   num_devices=N_CORES)
    xt = [nc.dram_tensor(f"xt{e}", [128, KC * T[e]], f32r, kind="ExternalInput")
          for e in range(NE)]
    if internal_wf:
        wf = [nc.dram_tensor(f"iwf{e}", [PATCH[e], 128, KC * D], f32r)
              for e in range(NE)]
    else:
        wf = [nc.dram_tensor(f"wf{e}", [PATCH[e], 128, KC * D], f32r,
                             kind="ExternalInput") for e in range(NE)]
    # partition-major layout [q, j, mt*D]: token t = mt*128 + j, so every
    # SBUF partition writes one contiguous run. Tail tokens >= T[e] are
    # garbage from the partial tile and are sliced off on the host.
    if internal_out:
        out = [nc.dram_tensor(f"io{e}", [PATCH[e], 128, NT[e] * D], f32)
               for e in range(NE)]
        tiny = nc.dram_tensor("tiny", [128, D], f32r, kind="ExternalOutput")
    else:
        out = [nc.dram_tensor(f"out{e}", [PATCH[e], 128, NT[e] * D], f32,
                              kind="ExternalOutput") for e in range(NE)]

    MAXNT = max(NT)
    with TileContext(nc) as tc:
        with (
            tc.tile_pool(name="xpool", bufs=1) as xpool,
            tc.tile_pool(name="wpool", bufs=wbufs) as wpool,
            tc.tile_pool(name="spool", bufs=sbufs) as spool,
            tc.tile_pool(name="ppool", bufs=8, space="PSUM") as ppool,
        ):
            xtiles = []
            for e in range(NE):
                t = xpool.tile([128, KC * T[e]], f32r, tag=f"xt{e}")
                nc.sync.dma_start(t[:], xt[e].ap())
                xtiles.append(t)

            # weights and output stores ride separate DMA rings so stores
            # never block weight prefetch (FIFO per ring).
            if rings == "a":      # w: sync | out: scalar+gpsimd
                w_engs = [nc.sync]
                out_engs = [nc.scalar, nc.gpsimd]
            else:                 # w: sync+scalar | out: gpsimd
                w_engs = [nc.sync, nc.scalar]
                out_engs = [nc.gpsimd]
            state = {"flip": 0, "oi": 0}

            def body():
                for e, q in _job_order(interleave):
                        wt = wpool.tile([128, KC * D], f32r, tag="wt")
                        w_engs[state["oi"] % len(w_engs)].dma_start(
                            wt[:], wf[e].ap()[q])
                        st = spool.tile([128, MAXNT * D], f32, tag="st")
                        for mt in range(NT[e]):
                            m = min(128, T[e] - 128 * mt)
                            ps = ppool.tile([128, D], psum_dt)
                            for k in range(KC):
                                nc.tensor.matmul(
                                    ps[:m, :],
                                    xtiles[e][:, k * T[e] + 128 * mt:
                                              k * T[e] + 128 * mt + m],
                                    wt[:, k * D:(k + 1) * D],
                                    start=(k == 0), stop=(k == KC - 1),
                                )
                            dst = st[:m, mt * D:(mt + 1) * D]
                            if copies == "dve" or state["flip"] % 2:
                                nc.vector.tensor_copy(dst, ps[:m, :])
                            else:
                                nc.scalar.copy(dst, ps[:m, :])
                            state["flip"] += 1
                        # one fully-contiguous DMA for this (e, q)
                        out_engs[state["oi"] % len(out_engs)].dma_start(
                            out[e].ap()[q], st[:, :NT[e] * D])
                        state["oi"] += 1

            if loop_n > 0:
                with tc.For_i(0, loop_n, 1):
                    body()
            else:
                body()
            if internal_out:
                nc.sync.dma_start(tiny.ap(), xtiles[0][:, :D])
    nc.compile()
    return nc


def _get_nc():
    if "nc" not in _CACHED:
        _CACHED["nc"] = _build_nc(dt=DT)
    return _CACHED["nc"]


def _prep(xs, Ws, gates, Wp, batch_index, expert_index, dt="f32r"):
    """Host-side shard prep. Returns (in_maps, row_of_expert, g_row)."""
    if dt == "f32r":
        cast = lambda a: a
    else:
        import ml_dtypes
        cast = lambda a: a.astype(ml_dtypes.bfloat16)
    row_of_expert = [np.nonzero(expert_index == e)[0] for e in range(NE)]
    g_row = gates[batch_index, expert_index].astype(np.float32)   # [NNZ]

    # Fused weights WF_e[q] = W_e[q*512:(q+1)*512, :].T @ Wp.T  -> [c, d_out];
    # device layout wf_e[q, p, k*512+d] with c = 128k + p.
    wf_in = []
    for e in range(NE):
        p = PATCH[e]
        w = Ws[e].reshape(p, D, D)                     # [q, d_mid, c]
        WF = np.einsum("qdc,od->qco", w, Wp, optimize=True)   # [q, c, d_out]
        wf_in.append(cast(np.ascontiguousarray(
            WF.reshape(p, KC, 128, D).transpose(0, 2, 1, 3)   # [q, p128, k, d]
              .reshape(p, 128, KC * D))))

    in_maps = []
    for c in range(N_CORES):
        m = {}
        for e in range(NE):
            rows = slice(c * ROWS_PER_CORE, (c + 1) * ROWS_PER_CORE)
            gr = g_row[row_of_expert[e][rows]]
            x = xs[e][rows] * gr[:, None, None]        # [16, L, 512]
            x = x.reshape(T[e], D)                     # tokens
            # xt[p, k*T + t] = x[t, 128k + p]
            m[f"xt{e}"] = cast(np.ascontiguousarray(
                x.reshape(T[e], KC, 128).transpose(2, 1, 0)
                 .reshape(128, KC * T[e])))
            m[f"wf{e}"] = wf_in[e]
        in_maps.append(m)
    return in_maps, row_of_expert, g_row


def _combine(results, row_of_expert, batch_index):
    """De-interleave q-major device outputs and gated-combine per batch."""
    combined = np.zeros((BATCH, SEQ, D), np.float32)
    for e in range(NE):
        p = PATCH[e]
        full = np.empty((ROWS_PER_EXPERT, SEQ, D), np.float32)
        for c in range(N_CORES):
            # device layout [q, j, mt, d]; token t = mt*128 + j
            raw = np.asarray(results[c][f"out{e}"], np.float32).reshape(
                p, 128, NT[e], D)
            dev = raw.transpose(0, 2, 1, 3).reshape(p, NT[e] * 128, D)[:, :T[e], :]
            # out_seq[r, l*p + q, :] = dev[q, r*L + l, :]
            blk = dev.reshape(p, ROWS_PER_CORE, L[e], D).transpose(1, 2, 0, 3)
            full[c * ROWS_PER_CORE:(c + 1) * ROWS_PER_CORE] = \
                blk.reshape(ROWS_PER_CORE, SEQ, D)
        bids = batch_index[row_of_expert[e]]
        if len(np.unique(bids)) == len(bids):
            combined[bids] += full
        else:
            np.add.at(combined, bids, full)
    return combined


# ---------------------------------------------------------------------------
# v2: expert/job-sharded, bf16, uniform micro-job SPMD program.
#
# Work unit: micro-job = one 512x512 fused weight applied to 1792 tokens
# (14 full 128-token tiles). Every (e, q) job splits into NXC[e] = L[e]/14
# micro-jobs; total 96 micro-jobs = 12 per core, identical on every core.
# Core r serves expert e = r//2, half h = r%2 (q in [h*p/2, (h+1)*p/2)).
# The core's x shard lives resident in SBUF (6 chunks of 1792 tokens,
# duplicated to 6 when the expert has fewer); micro-job j reads chunk j%6
# and weight slot j (host-duplicated per slot). In-loop HBM traffic per
# core: 6 MB weights + 22 MB outputs (bf16), ~4x less than v1 -- this
# matters because the 8 cores contend for ~1.7 TB/s aggregate DMA.
# ---------------------------------------------------------------------------
CHUNK = 1792                      # tokens per micro-job (14 tiles of 128)
NMJ = 12                          # micro-jobs per core
MT = CHUNK // 128                 # 14 token tiles per micro-job
NXC = [l // (CHUNK // 128) for l in L]    # distinct x chunks/core: [6,3,2,1]
XW = KC * CHUNK                   # sbuf width of one x chunk (7168)


def _build_nc2(loop_n=0, internal_w=False, internal_out=False,
               wbufs=4, sbufs=3):
    import concourse.mybir as mybir
    from concourse import bacc
    from concourse.tile import TileContext

    bf16 = mybir.dt.bfloat16
    f32 = mybir.dt.float32

    nc = bacc.Bacc("TRN2", target_bir_lowering=False, debug=False,
                   num_devices=N_CORES)
    xin = nc.dram_tensor("xin", [128, 6 * XW], bf16, kind="ExternalInput")
    if internal_w:
        win = nc.dram_tensor("iwin", [NMJ, 128, KC * D], bf16)
    else:
        win = nc.dram_tensor("win", [NMJ, 128, KC * D], bf16,
                             kind="ExternalInput")
    if internal_out:
        out = nc.dram_tensor("iout", [NMJ, 128, MT * D], bf16)
        tiny = nc.dram_tensor("tiny", [128, D], bf16, kind="ExternalOutput")
    else:
        out = nc.dram_tensor("out", [NMJ, 128, MT * D], bf16,
                             kind="ExternalOutput")

    with TileContext(nc) as tc:
        with (
            tc.tile_pool(name="xpool", bufs=1) as xpool,
            tc.tile_pool(name="wpool", bufs=wbufs) as wpool,
            tc.tile_pool(name="spool", bufs=sbufs) as spool,
            tc.tile_pool(name="ppool", bufs=8, space="PSUM") as ppool,
        ):
            xt = xpool.tile([128, 6 * XW], bf16, tag="xt")
            for c in range(6):
                (nc.sync if c % 2 else nc.scalar).dma_start(
                    xt[:, c * XW:(c + 1) * XW],
                    xin.ap()[:, c * XW:(c + 1) * XW])

            def body():
                for j in range(NMJ):
                    wt = wpool.tile([128, KC * D], bf16, tag="wt")
                    (nc.sync if j % 2 else nc.scalar).dma_start(
                        wt[:], win.ap()[j])
                    st = spool.tile([128, MT * D], bf16, tag="st")
                    xbase = (j % 6) * XW
                    for mt in range(MT):
                        ps = ppool.tile([128, D], f32)
                        for k in range(KC):
                            nc.tensor.matmul(
                                ps[:, :],
                                xt[:, xbase + k * CHUNK + mt * 128:
                                   xbase + k * CHUNK + mt * 128 + 128],
                                wt[:, k * D:(k + 1) * D],
                                start=(k == 0), stop=(k == KC - 1),
                            )
                        dst = st[:, mt * D:(mt + 1) * D]
                        if (j * MT + mt) % 2:
                            nc.vector.tensor_copy(dst, ps[:, :])
                        else:
                            nc.scalar.copy(dst, ps[:, :])
                    nc.gpsimd.dma_start(out.ap()[j], st[:])

            if loop_n > 0:
                with tc.For_i(0, loop_n, 1):
                    body()
            elif loop_n < 0:
                for _ in range(-loop_n):     # python-unrolled (sim ablations)
                    body()
            else:
                body()
            if internal_out:
                nc.sync.dma_start(tiny.ap(), xt[:, :D])
    nc.compile()
    return nc


def _prep2(xs, Ws, gates, Wp, batch_index, expert_index):
    """Per-core in_maps for v2. Returns (in_maps, row_of_expert, g_row)."""
    import ml_dtypes
    bf16 = ml_dtypes.bfloat16
    row_of_expert = [np.nonzero(expert_index == e)[0] for e in range(NE)]
    g_row = gates[batch_index, expert_index].astype(np.float32)

    wf_dev = []
    for e in range(NE):
        p = PATCH[e]
        w = Ws[e].reshape(p, D, D)
        WF = np.einsum("qdc,od->qco", w, Wp, optimize=True)    # [q, c, d_out]
        wf_dev.append(WF.reshape(p, KC, 128, D).transpose(0, 2, 1, 3)
                        .reshape(p, 128, KC * D).astype(bf16))

    in_maps = []
    for r in range(N_CORES):
        e, h = r // 2, r % 2
        nx, p = NXC[e], PATCH[e]
        gr = g_row[row_of_expert[e]]
        toks = (xs[e] * gr[:, None, None]).reshape(128 * L[e], D)
        xin = np.empty((128, 6 * XW), np.float32)
        for c in range(6):
            part = toks[(c % nx) * CHUNK:(c % nx + 1) * CHUNK]   # [1792, 512]
            xin[:, c * XW:(c + 1) * XW] = (
                part.reshape(CHUNK, KC, 128).transpose(2, 1, 0)
                    .reshape(128, XW))
        win = np.empty((NMJ, 128, KC * D), bf16)
        for j in range(NMJ):
            win[j] = wf_dev[e][h * (p // 2) + j // nx]
        in_maps.append({"xin": xin.astype(bf16), "win": win})
    return in_maps, row_of_expert, g_row


def _combine2(results, row_of_expert, batch_index):
    combined = np.zeros((BATCH, SEQ, D), np.float32)
    for e in range(NE):
        p = PATCH[e]
        full = np.empty((ROWS_PER_EXPERT, SEQ, D), np.float32)
        for h in range(2):
            r = 2 * e + h
            nx = NXC[e]
            O = np.asarray(results[r]["out"], np.float32) \
                  .reshape(NMJ, 128, MT, D).transpose(0, 2, 1, 3) \
                  .reshape(NMJ * CHUNK, D)
            for qi in range(p // 2):
                q = h * (p // 2) + qi
                blk = O[qi * nx * CHUNK:(qi + 1) * nx * CHUNK]
                full[:, q::p, :] = blk.reshape(ROWS_PER_EXPERT, L[e], D)
        bids = batch_index[row_of_expert[e]]
        if len(np.unique(bids)) == len(bids):
            combined[bids] += full
        else:
            np.add.at(combined, bids, full)
    return combined


def kernel(xs0, xs1, xs2, xs3, gates, W0, b0, W1, b1, W2, b2, W3, b3, Wp, bp,
           batch_index, expert_index, _collect_results=None):
    from concourse.bass_utils import run_bass_kernel_spmd

    xs = [np.asarray(x, np.float32) for x in (xs0, xs1, xs2, xs3)]
    Ws = [np.asarray(w, np.float32) for w in (W0, W1, W2, W3)]
    bs = [np.asarray(b, np.float32) for b in (b0, b1, b2, b3)]
    gates = np.asarray(gates, np.float32)
    Wp = np.asarray(Wp, np.float32)
    bp = np.asarray(bp, np.float32)
    batch_index = np.asarray(batch_index)
    expert_index = np.asarray(expert_index)

    if V == 2:
        in_maps, row_of_expert, g_row = _prep2(xs, Ws, gates, Wp,
                                               batch_index, expert_index)
        if "nc2" not in _CACHED:
            _CACHED["nc2"] = _build_nc2()
        nc = _CACHED["nc2"]
        res = run_bass_kernel_spmd(nc, in_maps, list(range(N_CORES)))
        if _collect_results is not None:
            _collect_results.append(res)
        combined = _combine2(res.results, row_of_expert, batch_index)
    else:
        in_maps, row_of_expert, g_row = _prep(xs, Ws, gates, Wp,
                                              batch_index, expert_index, dt=DT)
        nc = _get_nc()
        res = run_bass_kernel_spmd(nc, in_maps, list(range(N_CORES)))
        if _collect_results is not None:
            _collect_results.append(res)
        combined = _combine(res.results, row_of_expert, batch_index)

    # Bias terms (zero in this problem's inputs; handled for correctness).
    if any(np.any(b) for b in bs) or np.any(bp):
        for e in range(NE):
            p = PATCH[e]
            bF = bs[e].reshape(p, D) @ Wp.T + bp       # [q, d_out]
            bias_seq = np.tile(bF, (L[e], 1)).reshape(SEQ, D)
            bids = batch_index[row_of_expert[e]]
            gr = g_row[row_of_expert[e]]
            contrib = gr[:, None, None] * bias_seq[None]
            if len(np.unique(bids)) == len(bids):
                combined[bids] += contrib
            else:
                np.add.at(combined, bids, contrib)

    return combined



# revision 24
# speedup vs baseline: 3.1174x; 2.4742x over previous
"""Trainium2 Bass kernel for nn_CrossScalePeriodicFeatureAggregator.

Reference computation (per expert e with patch size p_e, L_e = 336 / p_e):
    h = einsum('nld,pd->nlp', xs_e, W_e) + b_e      # [128, L_e, p_e*512]
    h -> reshape [128, 336, 512]                     # seq-stitch
    proj = h @ Wp.T + bp                             # shared projection
    out[batch_index] += gate * proj                  # gated scatter-combine

Kernel strategy (V=2, 8 cores, SPMD, all-bf16):
  * Algebraic fusion: the two chained matmuls collapse into one. For output
    position s = l*p_e + q:  out[n, s, :] = x[n, l, :] @ WF_e[q]  where
    WF_e[q] = W_e[q*512:(q+1)*512, :].T @ Wp.T   (precomputed on host).
    This halves device FLOPs (90 GF instead of 180 GF).
  * Gates are folded into x rows on host (mathematically identical).
  * Expert sharding (v2): core r serves expert r//2, q-half r%2. Work is cut
    into 96 uniform micro-jobs (one 512x512 weight x 1792 tokens = 14 full
    128-token tiles), 12 per core -- a single uniform SPMD program with zero
    tile padding. Each core reads only its own 12 weight slots (6 MB bf16)
    instead of all 48 MB: the 8 cores contend for ~1.7 TB/s aggregate DMA,
    so total bytes, not per-core bytes, are what matter.
  * The core's x shard (<= 10752 tokens) stays resident in SBUF as 6 chunks
    of 1792 tokens (duplicated cyclically for experts with fewer); micro-job
    j reads chunk j%6 and weight slot j (host-duplicated), which makes the
    instruction stream identical across cores.
  * bf16 x/weights/outputs, f32 PSUM accumulation: same PE rate as fp32r
    (1 cyc/row at 512 free) but half the DMA bytes; measured max-abs-rel
    ~3.5e-3 (gate 2e-2).
  * PSUM pairing: 2-bank [128, 1024] PSUM tiles take two 4-matmul
    accumulation groups and evict with a single copy (fewer PSUM reads,
    ~5% faster than per-bank eviction).
  * Host de-interleaves q-strided positions and does the gated combine.
"""
import numpy as np

PATCH = [4, 8, 12, 24]
SEQ = 336
D = 512
NE = 4
BATCH = 256
ROWS_PER_EXPERT = 128
N_CORES = 8
ROWS_PER_CORE = ROWS_PER_EXPERT // N_CORES          # 16
L = [SEQ // p for p in PATCH]                       # [84, 42, 28, 14]
T = [ROWS_PER_CORE * l for l in L]                  # tokens/core: [1344, 672, 448, 224]
NT = [(t + 127) // 128 for t in T]                  # token tiles: [11, 6, 4, 2]
KC = 4                                              # contraction chunks of 128
DT = "f32r"                                         # "f32r" | "bf16" device dtype (v1)
V = 2                                               # active kernel version

_CACHED = {}


def _job_order(interleave):
    if not interleave:
        return [(e, q) for e in range(NE) for q in range(PATCH[e])]
    # merge experts by fractional position: pairs DMA-heavy (e3) jobs with
    # compute-heavy (e0) jobs to smooth HBM read demand
    jobs = [((q + 0.5) / PATCH[e], e, q)
            for e in range(NE) for q in range(PATCH[e])]
    return [(e, q) for _, e, q in sorted(jobs)]


def _build_nc(loop_n=0, internal_wf=False, internal_out=False,
              wbufs=8, sbufs=4, interleave=True, copies="dve", rings="a",
              dt="f32r", out_dt=None):
    """loop_n>0 wraps the compute body in a hardware For_i loop (differential
    HW timing); internal_wf/internal_out source weights from / sink outputs to
    internal DRAM so host transfer stays tiny (timing builds only)."""
    import concourse.mybir as mybir
    from concourse import bacc
    from concourse.tile import TileContext

    f32r = mybir.dt.float32r if dt == "f32r" else mybir.dt.bfloat16
    f32 = mybir.dt.float32 if (out_dt or dt) == "f32r" else mybir.dt.bfloat16
    psum_dt = mybir.dt.float32

    nc = bacc.Bacc("TRN2", target_bir_lowering=False, debug=False,
                   num_devices=N_CORES)
    xt = [nc.dram_tensor(f"xt{e}", [128, KC * T[e]], f32r, kind="ExternalInput")
          for e in range(NE)]
    if internal_wf:
        wf = [nc.dram_tensor(f"iwf{e}", [PATCH[e], 128, KC * D], f32r)
              for e in range(NE)]
    else:
        wf = [nc.dram_tensor(f"wf{e}", [PATCH[e], 128, KC * D], f32r,
                             kind="ExternalInput") for e in range(NE)]
    # partition-major layout [q, j, mt*D]: token t = mt*128 + j, so every
    # SBUF partition writes one contiguous run. Tail tokens >= T[e] are
    # garbage from the partial tile and are sliced off on the host.
    if internal_out:
        out = [nc.dram_tensor(f"io{e}", [PATCH[e], 128, NT[e] * D], f32)
               for e in range(NE)]
        tiny = nc.dram_tensor("tiny", [128, D], f32r, kind="ExternalOutput")
    else:
        out = [nc.dram_tensor(f"out{e}", [PATCH[e], 128, NT[e] * D], f32,
                              kind="ExternalOutput") for e in range(NE)]

    MAXNT = max(NT)
    with TileContext(nc) as tc:
        with (
            tc.tile_pool(name="xpool", bufs=1) as xpool,
            tc.tile_pool(name="wpool", bufs=wbufs) as wpool,
            tc.tile_pool(name="spool", bufs=sbufs) as spool,
            tc.tile_pool(name="ppool", bufs=8, space="PSUM") as ppool,
        ):
            xtiles = []
            for e in range(NE):
                t = xpool.tile([128, KC * T[e]], f32r, tag=f"xt{e}")
                nc.sync.dma_start(t[:], xt[e].ap())
                xtiles.append(t)

            # weights and output stores ride separate DMA rings so stores
            # never block weight prefetch (FIFO per ring).
            if rings == "a":      # w: sync | out: scalar+gpsimd
                w_engs = [nc.sync]
                out_engs = [nc.scalar, nc.gpsimd]
            else:                 # w: sync+scalar | out: gpsimd
                w_engs = [nc.sync, nc.scalar]
                out_engs = [nc.gpsimd]
            state = {"flip": 0, "oi": 0}

            def body():
                for e, q in _job_order(interleave):
                        wt = wpool.tile([128, KC * D], f32r, tag="wt")
                        w_engs[state["oi"] % len(w_engs)].dma_start(
                            wt[:], wf[e].ap()[q])
                        st = spool.tile([128, MAXNT * D], f32, tag="st")
                        for mt in range(NT[e]):
                            m = min(128, T[e] - 128 * mt)
                            ps = ppool.tile([128, D], psum_dt)
                            for k in range(KC):
                                nc.tensor.matmul(
                                    ps[:m, :],
                                    xtiles[e][:, k * T[e] + 128 * mt:
                                              k * T[e] + 128 * mt + m],
                                    wt[:, k * D:(k + 1) * D],
                                    start=(k == 0), stop=(k == KC - 1),
                                )
                            dst = st[:m, mt * D:(mt + 1) * D]
                            if copies == "dve" or state["flip"] % 2:
                                nc.vector.tensor_copy(dst, ps[:m, :])
                            else:
                                nc.scalar.copy(dst, ps[:m, :])
                            state["flip"] += 1
                        # one fully-contiguous DMA for this (e, q)
                        out_engs[state["oi"] % len(out_engs)].dma_start(
                            out[e].ap()[q], st[:, :NT[e] * D])
                        state["oi"] += 1

            if loop_n > 0:
                with tc.For_i(0, loop_n, 1):
                    body()
            else:
                body()
            if internal_out:
                nc.sync.dma_start(tiny.ap(), xtiles[0][:, :D])
    nc.compile()
    return nc


def _get_nc():
    if "nc" not in _CACHED:
        _CACHED["nc"] = _build_nc(dt=DT)
    return _CACHED["nc"]


def _prep(xs, Ws, gates, Wp, batch_index, expert_index, dt="f32r"):
    """Host-side shard prep. Returns (in_maps, row_of_expert, g_row)."""
    if dt == "f32r":
        cast = lambda a: a
    else:
        import ml_dtypes
        cast = lambda a: a.astype(ml_dtypes.bfloat16)
    row_of_expert = [np.nonzero(expert_index == e)[0] for e in range(NE)]
    g_row = gates[batch_index, expert_index].astype(np.float32)   # [NNZ]

    # Fused weights WF_e[q] = W_e[q*512:(q+1)*512, :].T @ Wp.T  -> [c, d_out];
    # device layout wf_e[q, p, k*512+d] with c = 128k + p.
    wf_in = []
    for e in range(NE):
        p = PATCH[e]
        w = Ws[e].reshape(p, D, D)                     # [q, d_mid, c]
        WF = np.einsum("qdc,od->qco", w, Wp, optimize=True)   # [q, c, d_out]
        wf_in.append(cast(np.ascontiguousarray(
            WF.reshape(p, KC, 128, D).transpose(0, 2, 1, 3)   # [q, p128, k, d]
              .reshape(p, 128, KC * D))))

    in_maps = []
    for c in range(N_CORES):
        m = {}
        for e in range(NE):
            rows = slice(c * ROWS_PER_CORE, (c + 1) * ROWS_PER_CORE)
            gr = g_row[row_of_expert[e][rows]]
            x = xs[e][rows] * gr[:, None, None]        # [16, L, 512]
            x = x.reshape(T[e], D)                     # tokens
            # xt[p, k*T + t] = x[t, 128k + p]
            m[f"xt{e}"] = cast(np.ascontiguousarray(
                x.reshape(T[e], KC, 128).transpose(2, 1, 0)
                 .reshape(128, KC * T[e])))
            m[f"wf{e}"] = wf_in[e]
        in_maps.append(m)
    return in_maps, row_of_expert, g_row


def _combine(results, row_of_expert, batch_index):
    """De-interleave q-major device outputs and gated-combine per batch."""
    combined = np.zeros((BATCH, SEQ, D), np.float32)
    for e in range(NE):
        p = PATCH[e]
        full = np.empty((ROWS_PER_EXPERT, SEQ, D), np.float32)
        for c in range(N_CORES):
            # device layout [q, j, mt, d]; token t = mt*128 + j
            raw = np.asarray(results[c][f"out{e}"], np.float32).reshape(
                p, 128, NT[e], D)
            dev = raw.transpose(0, 2, 1, 3).reshape(p, NT[e] * 128, D)[:, :T[e], :]
            # out_seq[r, l*p + q, :] = dev[q, r*L + l, :]
            blk = dev.reshape(p, ROWS_PER_CORE, L[e], D).transpose(1, 2, 0, 3)
            full[c * ROWS_PER_CORE:(c + 1) * ROWS_PER_CORE] = \
                blk.reshape(ROWS_PER_CORE, SEQ, D)
        bids = batch_index[row_of_expert[e]]
        if len(np.unique(bids)) == len(bids):
            combined[bids] += full
        else:
            np.add.at(combined, bids, full)
    return combined


# ---------------------------------------------------------------------------
# v2: expert/job-sharded, bf16, uniform micro-job SPMD program.
#
# Work unit: micro-job = one 512x512 fused weight applied to 1792 tokens
# (14 full 128-token tiles). Every (e, q) job splits into NXC[e] = L[e]/14
# micro-jobs; total 96 micro-jobs = 12 per core, identical on every core.
# Core r serves expert e = r//2, half h = r%2 (q in [h*p/2, (h+1)*p/2)).
# The core's x shard lives resident in SBUF (6 chunks of 1792 tokens,
# duplicated to 6 when the expert has fewer); micro-job j reads chunk j%6
# and weight slot j (host-duplicated per slot). In-loop HBM traffic per
# core: 6 MB weights + 22 MB outputs (bf16), ~4x less than v1 -- this
# matters because the 8 cores contend for ~1.7 TB/s aggregate DMA.
# ---------------------------------------------------------------------------
CHUNK = 1792                      # tokens per micro-job (14 tiles of 128)
NMJ = 12                          # micro-jobs per core
MT = CHUNK // 128                 # 14 token tiles per micro-job
NXC = [l // (CHUNK // 128) for l in L]    # distinct x chunks/core: [6,3,2,1]
XW = KC * CHUNK                   # sbuf width of one x chunk (7168)


def _build_nc2(loop_n=0, internal_w=False, internal_out=False,
               wbufs=4, sbufs=3, internal_x=False, psum_pair=True,
               w_engs="ss", pair_jobs=False, evict="s"):
    import concourse.mybir as mybir
    from concourse import bacc
    from concourse.tile import TileContext

    bf16 = mybir.dt.bfloat16
    f32 = mybir.dt.float32

    nc = bacc.Bacc("TRN2", target_bir_lowering=False, debug=False,
                   num_devices=N_CORES)
    if internal_x:
        xin = nc.dram_tensor("ixin", [128, 6 * XW], bf16)
    else:
        xin = nc.dram_tensor("xin", [128, 6 * XW], bf16, kind="ExternalInput")
    if internal_w:
        win = nc.dram_tensor("iwin", [NMJ, 128, KC * D], bf16)
    else:
        win = nc.dram_tensor("win", [NMJ, 128, KC * D], bf16,
                             kind="ExternalInput")
    if internal_out:
        out = nc.dram_tensor("iout", [NMJ, 128, MT * D], bf16)
        tiny = nc.dram_tensor("tiny", [128, D], bf16, kind="ExternalOutput")
    else:
        out = nc.dram_tensor("out", [NMJ, 128, MT * D], bf16,
                             kind="ExternalOutput")

    with TileContext(nc) as tc:
        with (
            tc.tile_pool(name="xpool", bufs=1) as xpool,
            tc.tile_pool(name="wpool", bufs=wbufs) as wpool,
            tc.tile_pool(name="spool", bufs=sbufs) as spool,
            tc.tile_pool(name="ppool", bufs=8, space="PSUM") as ppool,
        ):
            xt = xpool.tile([128, 6 * XW], bf16, tag="xt")
            for c in range(6):
                (nc.sync if c % 2 else nc.scalar).dma_start(
                    xt[:, c * XW:(c + 1) * XW],
                    xin.ap()[:, c * XW:(c + 1) * XW])

            wq = {"ss": [nc.sync, nc.scalar], "s": [nc.sync],
                  "g": [nc.gpsimd]}[w_engs]

            def evict_copy(dst, src, parity):
                if evict == "v" or (evict == "alt" and parity):
                    nc.vector.tensor_copy(dst, src)
                else:
                    nc.scalar.copy(dst, src)

            def body_paired():
                # job pairs (j, j+6) share x chunk j: consecutive matmuls
                # reuse the same lhsT (stationary) slice across two weights.
                for j in range(6):
                    wtA = wpool.tile([128, KC * D], bf16, tag="wt",
                                     name="wtA")
                    wq[0].dma_start(wtA[:], win.ap()[j])
                    wtB = wpool.tile([128, KC * D], bf16, tag="wt",
                                     name="wtB")
                    wq[-1].dma_start(wtB[:], win.ap()[j + 6])
                    stA = spool.tile([128, MT * D], bf16, tag="st",
                                     name="stA")
                    stB = spool.tile([128, MT * D], bf16, tag="st",
                                     name="stB")
                    xbase = j * XW
                    for mt in range(MT):
                        psA = ppool.tile([128, D], f32, name="psA",
                                         tag="psA", bufs=4)
                        psB = ppool.tile([128, D], f32, name="psB",
                                         tag="psB", bufs=4)
                        for k in range(KC):
                            xs = xt[:, xbase + k * CHUNK + mt * 128:
                                    xbase + k * CHUNK + mt * 128 + 128]
                            nc.tensor.matmul(psA[:, :], xs,
                                             wtA[:, k * D:(k + 1) * D],
                                             start=(k == 0),
                                             stop=(k == KC - 1))
                            nc.tensor.matmul(psB[:, :], xs,
                                             wtB[:, k * D:(k + 1) * D],
                                             start=(k == 0),
                                             stop=(k == KC - 1))
                        dA = stA[:, mt * D:(mt + 1) * D]
                        dB = stB[:, mt * D:(mt + 1) * D]
                        if mt % 2:
                            nc.vector.tensor_copy(dA, psA[:, :])
                            nc.scalar.copy(dB, psB[:, :])
                        else:
                            nc.scalar.copy(dA, psA[:, :])
                            nc.vector.tensor_copy(dB, psB[:, :])
                    nc.gpsimd.dma_start(out.ap()[j], stA[:])
                    nc.gpsimd.dma_start(out.ap()[j + 6], stB[:])

            def body():
                for j in range(NMJ):
                    wt = wpool.tile([128, KC * D], bf16, tag="wt")
                    wq[j % len(wq)].dma_start(wt[:], win.ap()[j])
                    st = spool.tile([128, MT * D], bf16, tag="st")
                    xbase = (j % 6) * XW

                    def mm(ps, pcol, mt):
                        for k in range(KC):
                            nc.tensor.matmul(
                                ps[:, pcol:pcol + D],
                                xt[:, xbase + k * CHUNK + mt * 128:
                                   xbase + k * CHUNK + mt * 128 + 128],
                                wt[:, k * D:(k + 1) * D],
                                start=(k == 0), stop=(k == KC - 1),
                            )

                    if psum_pair:
                        # 2-bank PSUM tiles: two matmul groups, one eviction
                        for mh in range(MT // 2):
                            ps = ppool.tile([128, 2 * D], f32, bufs=4,
                                            name="ps2")
                            mm(ps, 0, 2 * mh)
                            mm(ps, D, 2 * mh + 1)
                            dst = st[:, 2 * mh * D:(2 * mh + 2) * D]
                            evict_copy(dst, ps[:, :], mh % 2)
                    else:
                        for mt in range(MT):
                            ps = ppool.tile([128, D], f32)
                            mm(ps, 0, mt)
                            dst = st[:, mt * D:(mt + 1) * D]
                            evict_copy(dst, ps[:, :], mt % 2)
                    nc.gpsimd.dma_start(out.ap()[j], st[:])

            fn = body_paired if pair_jobs else body
            if loop_n > 0:
                with tc.For_i(0, loop_n, 1):
                    fn()
            elif loop_n < 0:
                for _ in range(-loop_n):     # python-unrolled (sim ablations)
                    fn()
            else:
                fn()
            if internal_out:
                nc.sync.dma_start(tiny.ap(), xt[:, :D])
    nc.compile()
    return nc


def _prep2(xs, Ws, gates, Wp, batch_index, expert_index):
    """Per-core in_maps for v2. Returns (in_maps, row_of_expert, g_row)."""
    import ml_dtypes
    bf16 = ml_dtypes.bfloat16
    row_of_expert = [np.nonzero(expert_index == e)[0] for e in range(NE)]
    g_row = gates[batch_index, expert_index].astype(np.float32)

    wf_dev = []
    for e in range(NE):
        p = PATCH[e]
        w = Ws[e].reshape(p, D, D)
        WF = np.einsum("qdc,od->qco", w, Wp, optimize=True)    # [q, c, d_out]
        wf_dev.append(WF.reshape(p, KC, 128, D).transpose(0, 2, 1, 3)
                        .reshape(p, 128, KC * D).astype(bf16))

    in_maps = []
    for r in range(N_CORES):
        e, h = r // 2, r % 2
        nx, p = NXC[e], PATCH[e]
        gr = g_row[row_of_expert[e]]
        toks = (xs[e] * gr[:, None, None]).reshape(128 * L[e], D)
        xin = np.empty((128, 6 * XW), np.float32)
        for c in range(6):
            part = toks[(c % nx) * CHUNK:(c % nx + 1) * CHUNK]   # [1792, 512]
            xin[:, c * XW:(c + 1) * XW] = (
                part.reshape(CHUNK, KC, 128).transpose(2, 1, 0)
                    .reshape(128, XW))
        win = np.empty((NMJ, 128, KC * D), bf16)
        for j in range(NMJ):
            win[j] = wf_dev[e][h * (p // 2) + j // nx]
        in_maps.append({"xin": xin.astype(bf16), "win": win})
    return in_maps, row_of_expert, g_row


def _combine2(results, row_of_expert, batch_index):
    combined = np.zeros((BATCH, SEQ, D), np.float32)
    for e in range(NE):
        p = PATCH[e]
        full = np.empty((ROWS_PER_EXPERT, SEQ, D), np.float32)
        for h in range(2):
            r = 2 * e + h
            nx = NXC[e]
            O = np.asarray(results[r]["out"], np.float32) \
                  .reshape(NMJ, 128, MT, D).transpose(0, 2, 1, 3) \
                  .reshape(NMJ * CHUNK, D)
            for qi in range(p // 2):
                q = h * (p // 2) + qi
                blk = O[qi * nx * CHUNK:(qi + 1) * nx * CHUNK]
                full[:, q::p, :] = blk.reshape(ROWS_PER_EXPERT, L[e], D)
        bids = batch_index[row_of_expert[e]]
        if len(np.unique(bids)) == len(bids):
            combined[bids] += full
        else:
            np.add.at(combined, bids, full)
    return combined


def kernel(xs0, xs1, xs2, xs3, gates, W0, b0, W1, b1, W2, b2, W3, b3, Wp, bp,
           batch_index, expert_index, _collect_results=None):
    from concourse.bass_utils import run_bass_kernel_spmd

    xs = [np.asarray(x, np.float32) for x in (xs0, xs1, xs2, xs3)]
    Ws = [np.asarray(w, np.float32) for w in (W0, W1, W2, W3)]
    bs = [np.asarray(b, np.float32) for b in (b0, b1, b2, b3)]
    gates = np.asarray(gates, np.float32)
    Wp = np.asarray(Wp, np.float32)
    bp = np.asarray(bp, np.float32)
    batch_index = np.asarray(batch_index)
    expert_index = np.asarray(expert_index)

    if V == 2:
        in_maps, row_of_expert, g_row = _prep2(xs, Ws, gates, Wp,
                                               batch_index, expert_index)
        if "nc2" not in _CACHED:
            _CACHED["nc2"] = _build_nc2()
        nc = _CACHED["nc2"]
        res = run_bass_kernel_spmd(nc, in_maps, list(range(N_CORES)))
        if _collect_results is not None:
            _collect_results.append(res)
        combined = _combine2(res.results, row_of_expert, batch_index)
    else:
        in_maps, row_of_expert, g_row = _prep(xs, Ws, gates, Wp,
                                              batch_index, expert_index, dt=DT)
        nc = _get_nc()
        res = run_bass_kernel_spmd(nc, in_maps, list(range(N_CORES)))
        if _collect_results is not None:
            _collect_results.append(res)
        combined = _combine(res.results, row_of_expert, batch_index)

    # Bias terms (zero in this problem's inputs; handled for correctness).
    if any(np.any(b) for b in bs) or np.any(bp):
        for e in range(NE):
            p = PATCH[e]
            bF = bs[e].reshape(p, D) @ Wp.T + bp       # [q, d_out]
            bias_seq = np.tile(bF, (L[e], 1)).reshape(SEQ, D)
            bids = batch_index[row_of_expert[e]]
            gr = g_row[row_of_expert[e]]
            contrib = gr[:, None, None] * bias_seq[None]
            if len(np.unique(bids)) == len(bids):
                combined[bids] += contrib
            else:
                np.add.at(combined, bids, contrib)

    return combined

